# revision 8
# baseline (speedup 1.0000x reference)
"""Trainium2 Bass kernel for nn_DenseCoordination (gnn_message_passing).

Math (per batch b):
    hi = s @ W1a ; hj = s @ W1b                       [N, 2D]
    q[i,j,:] = (s_i * s_j) @ W1c + hi_i + hj_j + b1   [N, N, 2D]
    logits[i,j] = relu(q[i,j,:]) @ W2 + b2
    w = softmax(mask(logits), axis=-1) (nan_to_num)
    ctx = w @ s ; gate = ones

Sharding: 8 cores = 4 batches x 2 i-halves. Each core owns b = c//2 and
i in [128*(c%2), 128*(c%2)+128), computes its [128, N] logits / w / ctx.

v3 changes vs v2 baseline:
  - pt (si*sj) production moved from DVE to GPSIMD (Pool engine), freeing
    DVE for the consume.
  - Setup matmuls (hi/hj) in bf16 directly; no more f32r conversion copies.
  - Consume split: DVE does pos cols via max-trick stt (relu(H+hi) =
    max(H,-hi)+hi, the +hi row-constant cancels in softmax); ACT does neg
    cols via Relu+accum on H+hi (hi added by an identity matmul on PE).
  - Tail in bf16 (w transpose + ctx matmul with bf16 operands).
  - Body unrolled x2 inside For_i with bufs=2 pools so iteration t+1's
    input DMA + setup overlaps iteration t's j-loop / tail.
"""

import sys

sys.path.insert(0, "/opt/trn_rl_repo")

import numpy as np
import ml_dtypes

import concourse.bacc as bacc
import concourse.bass as bass
import concourse.tile as tile
from concourse import mybir
from concourse.bass_utils import run_bass_kernel_spmd

D = 256
N = 256
B = 4
H2 = 512  # 2*D
NI = 128  # i rows per core
N_CORES = 8
F32 = mybir.dt.float32
BF16 = mybir.dt.bfloat16
NEG_BIG = -1.0e30

_BUILD_CACHE: dict = {}


def _build(npos: int, with_loop: bool = True):
    AF = mybir.ActivationFunctionType
    ALU = mybir.AluOpType
    # Columns are permuted pos-first: [0:npos) pos, [npos:H2) neg.
    # ACT takes the SMALLER block (it needs hi pre-added by a PE identity
    # matmul, which costs PE cycles proportional to its width); DVE takes
    # the larger block via the max-trick stt (no hi add needed: the +hi
    # row-constant cancels in softmax).
    act_on_pos = npos <= H2 - npos
    if act_on_pos:
        aLo, aHi = 0, npos          # ACT: relu+accum over pos cols
        dLo, dHi = npos, H2         # DVE: max-trick over neg cols
    else:
        aLo, aHi = npos, H2         # ACT: relu+accum over neg cols
        dLo, dHi = 0, npos          # DVE: max-trick over pos cols
    cA = aHi - aLo
    cD = dHi - dLo

    nc = bacc.Bacc("TRN2", target_bir_lowering=False, debug=False,
                   num_devices=N_CORES)

    sT_in = nc.dram_tensor("sT", [D, N], F32, kind="ExternalInput").ap()
    sb_in = nc.dram_tensor("sb", [N, D], BF16, kind="ExternalInput").ap()
    sTb_in = nc.dram_tensor("sTb", [D, N], BF16, kind="ExternalInput").ap()
    sTib_in = nc.dram_tensor("sTib", [D, NI], BF16, kind="ExternalInput").ap()
    w1ab_in = nc.dram_tensor("W1ab", [D, H2], BF16, kind="ExternalInput").ap()
    w1bb_in = nc.dram_tensor("W1bb", [D, H2], BF16, kind="ExternalInput").ap()
    w1cb_in = nc.dram_tensor("W1cb", [D, H2], BF16, kind="ExternalInput").ap()
    b1b_in = nc.dram_tensor("b1b", [1, H2], BF16, kind="ExternalInput").ap()
    madd_in = nc.dram_tensor("madd", [NI, N], F32, kind="ExternalInput").ap()
    rowind_in = nc.dram_tensor("rowind", [NI, 1], F32, kind="ExternalInput").ap()
    identb_in = nc.dram_tensor("identb", [128, 128], BF16,
                               kind="ExternalInput").ap()
    nrep_in = nc.dram_tensor("nrep", [1, 1], mybir.dt.int32,
                             kind="ExternalInput").ap()
    w_out = nc.dram_tensor("w", [NI, N], F32, kind="ExternalOutput").ap()
    ctx_out = nc.dram_tensor("ctx", [NI, D], F32, kind="ExternalOutput").ap()

    with tile.TileContext(nc) as tc:
        with (
            tc.tile_pool(name="persist", bufs=2) as pp,
            tc.tile_pool(name="prod", bufs=6) as prodp,
            tc.tile_pool(name="trash", bufs=4) as trp,
            tc.tile_pool(name="psum", bufs=6, space="PSUM") as psp,
            tc.tile_pool(name="psum2", bufs=1, space="PSUM") as psp2,
            tc.tile_pool(name="psum3", bufs=1, space="PSUM") as psp3,
            tc.tile_pool(name="small", bufs=4) as smp,
        ):
            def body():
                # ---- load inputs into SBUF
                sT_sb = []
                sb_sb = []
                sTb_sb = []
                sTib_sb = []
                w1ab_sb = []
                w1bb_sb = []
                w1cb_sb = []
                for c in range(2):
                    t = pp.tile([128, N], F32, tag=f"sT{c}", name=f"sT{c}")
                    nc.sync.dma_start(t[:], sT_in[128 * c:128 * c + 128, :])
                    sT_sb.append(t)
                    t = pp.tile([128, D], BF16, tag=f"sb{c}", name=f"sb{c}")
                    nc.sync.dma_start(t[:], sb_in[128 * c:128 * c + 128, :])
                    sb_sb.append(t)
                    t = pp.tile([128, N], BF16, tag=f"sTb{c}", name=f"sTb{c}")
                    nc.sync.dma_start(t[:], sTb_in[128 * c:128 * c + 128, :])
                    sTb_sb.append(t)
                    t = pp.tile([128, NI], BF16, tag=f"sTib{c}", name=f"sTib{c}")
                    nc.sync.dma_start(t[:], sTib_in[128 * c:128 * c + 128, :])
                    sTib_sb.append(t)
                    for nm, src, lst in (("a", w1ab_in, w1ab_sb),
                                         ("b", w1bb_in, w1bb_sb),
                                         ("c", w1cb_in, w1cb_sb)):
                        t = pp.tile([128, H2], BF16, tag=f"W1{nm}{c}",
                                    name=f"W1{nm}{c}")
                        nc.sync.dma_start(t[:], src[128 * c:128 * c + 128, :])
                        lst.append(t)
                b1_sb = pp.tile([1, H2], BF16, tag="b1")
                nc.sync.dma_start(b1_sb[:], b1b_in[:])
                madd_sb = pp.tile([NI, N], F32, tag="madd")
                nc.sync.dma_start(madd_sb[:], madd_in[:])
                rowind_sb = pp.tile([NI, 1], F32, tag="rowind")
                nc.sync.dma_start(rowind_sb[:], rowind_in[:])
                identb_sb = pp.tile([128, 128], BF16, tag="identb")
                nc.sync.dma_start(identb_sb[:], identb_in[:])
                onesb_sb = pp.tile([1, 128], BF16, tag="onesb")
                nc.gpsimd.memset(onesb_sb[:], 1.0)

                # ---- setup: HJ' = s @ W1b' + b1'  (2 chunks of 128 j's)
                hj_bf = []
                for jc in range(2):
                    ps = psp.tile([128, H2], F32, tag="ps")
                    for kc in range(2):
                        nc.tensor.matmul(
                            ps[:], sTb_sb[kc][:, 128 * jc:128 * jc + 128],
                            w1bb_sb[kc][:], start=(kc == 0), stop=False)
                    nc.tensor.matmul(ps[:], onesb_sb[0:1, :], b1_sb[0:1, :],
                                     start=False, stop=True)
                    t = pp.tile([128, H2], BF16, tag=f"hjb{jc}", name=f"hjb{jc}")
                    nc.scalar.copy(t[:], ps[:])
                    hj_bf.append(t)

                # ---- setup: HI' = s[i-range] @ W1a' (no b1)
                ps = psp.tile([128, H2], F32, tag="ps")
                for kc in range(2):
                    nc.tensor.matmul(ps[:], sTib_sb[kc][:], w1ab_sb[kc][:],
                                     start=(kc == 0), stop=(kc == 1))
                hi_bf = pp.tile([128, H2], BF16, tag="hib")
                nc.scalar.copy(hi_bf[:], ps[:])
                neghi_sb = pp.tile([128, cD], F32, tag="neghi")
                nc.scalar.mul(neghi_sb[:], ps[:, dLo:dHi], -1.0)
                # (the +hi shift in the max-trick is constant per row i over
                #  the DVE block -> cancels in softmax)

                # ---- main fixed-j loop, software-pipelined.
                # pt production runs LA iterations ahead of the matmuls so
                # the (in-order) DVE never makes the PE wait for its
                # stationary operand: per-j DVE order is
                # [pt_{j+LA}, stt_j], not [pt_j, stt_j].
                accp = pp.tile([NI, N], F32, tag="accp")
                acca = pp.tile([NI, N], F32, tag="acca")
                LA = 3
                pt_ring = {}

                def emit_pt(j):
                    pts = []
                    for kc in range(2):
                        t = prodp.tile([128, NI], BF16, tag=f"pt{kc}",
                                       name=f"pt{kc}")
                        nc.vector.tensor_scalar_mul(
                            t[:], sTib_sb[kc][:],
                            sT_sb[kc][:, j:j + 1])
                        pts.append(t)
                    pt_ring[j] = pts

                for j in range(LA):
                    emit_pt(j)
                for j in range(N):
                    jc, jr = j // 128, j % 128
                    if j + LA < N:
                        emit_pt(j + LA)
                    pt = pt_ring.pop(j)
                    ps = psp.tile([128, H2], F32, tag="ps")
                    for kc in range(2):
                        nc.tensor.matmul(
                            ps[:], pt[kc][:], w1cb_sb[kc][:],
                            start=(kc == 0), stop=False)
                    # += hj'_j + b1' on all cols (one-hot row jr broadcast)
                    nc.tensor.matmul(
                        ps[:],
                        identb_sb[:, jr:jr + 1].to_broadcast((128, 128)),
                        hj_bf[jc][:],
                        start=False, stop=(cA == 0))
                    if cA > 0:
                        # += hi' on the ACT cols only
                        nc.tensor.matmul(
                            ps[:, aLo:aHi], identb_sb[:], hi_bf[:, aLo:aHi],
                            start=False, stop=True)
                        tr2 = trp.tile([128, cA], BF16, tag="tr2")
                        nc.scalar.activation(tr2[:], ps[:, aLo:aHi], AF.Relu,
                                             accum_out=acca[:, j:j + 1])
                    else:
                        nc.gpsimd.memset(acca[:, j:j + 1], 0.0)
                    if cD > 0:
                        tr1 = trp.tile([128, cD], BF16, tag="tr1")
                        nc.vector.scalar_tensor_tensor(
                            out=tr1[:], in0=ps[:, dLo:dHi], scalar=0.0,
                            in1=neghi_sb[:], op0=ALU.add,
                            op1=ALU.max, accum_out=accp[:, j:j + 1])
                    else:
                        nc.gpsimd.memset(accp[:, j:j + 1], 0.0)

                # logits: pos-block accumulator minus neg-block accumulator
                # (per-row-i constants cancel in softmax)
                logits = pp.tile([NI, N], F32, tag="logits")
                if act_on_pos:
                    nc.vector.tensor_sub(logits[:], acca[:], accp[:])
                else:
                    nc.vector.tensor_sub(logits[:], accp[:], acca[:])

                # ---- masked softmax over j
                l2 = pp.tile([NI, N], F32, tag="l2")
                nc.vector.tensor_add(l2[:], logits[:], madd_sb[:])
                negm = smp.tile([NI, 1], F32, tag="negm")
                nc.vector.tensor_reduce(negm[:], l2[:],
                                        axis=mybir.AxisListType.X, op=ALU.max,
                                        negate=True)
                ex = pp.tile([NI, N], F32, tag="ex")
                ssum = smp.tile([NI, 1], F32, tag="ssum")
                nc.scalar.activation(ex[:], l2[:], AF.Exp, bias=negm[:, 0:1],
                                     accum_out=ssum[:, 0:1])
                rec = smp.tile([NI, 1], F32, tag="rec")
                nc.vector.reciprocal(rec[:], ssum[:])
                rec2 = smp.tile([NI, 1], F32, tag="rec2")
                nc.vector.tensor_mul(rec2[:], rec[:], rowind_sb[:])
                w_sb = pp.tile([NI, N], F32, tag="wsb")
                nc.vector.tensor_scalar_mul(w_sb[:], ex[:], rec2[:, 0:1])
                wb_sb = pp.tile([NI, N], BF16, tag="wbsb")
                nc.vector.tensor_copy(wb_sb[:], w_sb[:])

                # ---- ctx = w @ s  (transpose w on the PE first, bf16)
                wt_sb = []
                for jc in range(2):
                    pst = psp3.tile([128, 128], BF16, tag="pst")
                    nc.tensor.transpose(pst[:],
                                        wb_sb[:, 128 * jc:128 * jc + 128],
                                        identb_sb[:])
                    t = smp.tile([128, 128], BF16, tag=f"wt{jc}", name=f"wt{jc}")
                    nc.vector.tensor_copy(t[:], pst[:])
                    wt_sb.append(t)
                psc = psp2.tile([128, D], F32, tag="tail")
                for jc in range(2):
                    nc.tensor.matmul(psc[:], wt_sb[jc][:], sb_sb[jc][:],
                                     start=(jc == 0), stop=(jc == 1))
                ctx_sb = pp.tile([NI, D], F32, tag="ctxsb")
                nc.scalar.copy(ctx_sb[:], psc[:])

                # ---- outputs
                nc.sync.dma_start(w_out[:], w_sb[:])
                nc.sync.dma_start(ctx_out[:], ctx_sb[:])

            if with_loop:
                nrep_sb = smp.tile([1, 1], mybir.dt.int32, tag="nrep")
                nc.sync.dma_start(nrep_sb[:], nrep_in[:])
                rv = nc.values_load(nrep_sb[0:1, 0:1], min_val=1,
                                    max_val=100000,
                                    skip_runtime_bounds_check=True)
                with tc.For_i(0, rv, 1):
                    body()
                    body()
            else:
                body()

    nc.compile()
    return nc


def _prep(s, W1, b1, W2, b2, adj_allowed, active_mask, act_mask):
    s = np.ascontiguousarray(np.asarray(s, dtype=np.float32))
    W1 = np.asarray(W1, dtype=np.float32)
    b1 = np.asarray(b1, dtype=np.float32).reshape(-1)
    W2 = np.asarray(W2, dtype=np.float32).reshape(-1)  # [2D]
    adj = np.asarray(adj_allowed)
    am = np.asarray(active_mask)
    km = np.asarray(act_mask)

    pos = W2 >= 0.0
    perm = np.concatenate([np.nonzero(pos)[0], np.nonzero(~pos)[0]])
    npos = int(pos.sum())
    w2p = np.abs(W2[perm])
    W1a = np.ascontiguousarray(W1[:D][:, perm] * w2p[None, :])
    W1b = np.ascontiguousarray(W1[D:2 * D][:, perm] * w2p[None, :])
    W1c = np.ascontiguousarray(W1[2 * D:][:, perm] * w2p[None, :])
    b1p = np.ascontiguousarray((b1[perm] * w2p)[None, :])

    valid = (adj > 0) & (am > 0)[:, None, :] & (km > 0)[:, :, None]
    madd = np.where(valid, np.float32(0.0), np.float32(NEG_BIG))
    rowind = valid.any(axis=-1).astype(np.float32)
    return s, W1a, W1b, W1c, b1p, madd, rowind, npos


def _in_maps(s, W1a, W1b, W1c, b1p, madd, rowind, nrep):
    # device trip count: body is unrolled x2 inside For_i
    trip = max(1, (int(nrep) + 1) // 2)
    nrep_arr = np.full((1, 1), trip, dtype=np.int32)
    bf = ml_dtypes.bfloat16
    w1ab = W1a.astype(bf)
    w1bb = W1b.astype(bf)
    w1cb = W1c.astype(bf)
    b1b = b1p.astype(bf)
    identb = np.eye(128, dtype=np.float32).astype(bf)
    maps = []
    for c in range(N_CORES):
        b, i0 = c // 2, NI * (c % 2)
        sb = s[b]
        sTb = np.ascontiguousarray(sb.T)
        sTb_bf = sTb.astype(bf)
        maps.append({
            "sT": sTb,
            "sb": sb.astype(bf),
            "sTb": sTb_bf,
            "sTib": np.ascontiguousarray(sTb_bf[:, i0:i0 + NI]),
            "W1ab": w1ab, "W1bb": w1bb, "W1cb": w1cb, "b1b": b1b,
            "madd": np.ascontiguousarray(madd[b, i0:i0 + NI]),
            "rowind": np.ascontiguousarray(rowind[b, i0:i0 + NI, None]),
            "identb": identb,
            "nrep": nrep_arr,
        })
    return maps


def _gather(results):
    w = np.empty((B, N, N), dtype=np.float32)
    ctx = np.empty((B, N, D), dtype=np.float32)
    for c in range(N_CORES):
        b, i0 = c // 2, NI * (c % 2)
        w[b, i0:i0 + NI] = results[c]["w"]
        ctx[b, i0:i0 + NI] = results[c]["ctx"]
    gate = np.ones((B, N, N), dtype=np.float32)
    return ctx, gate, w


def _get_program(npos, with_loop=True):
    key = (npos, with_loop)
    if key not in _BUILD_CACHE:
        _BUILD_CACHE[key] = _build(npos, with_loop=with_loop)
    return _BUILD_CACHE[key]


def run(nrep, *, with_loop=True, **inputs):
    """Run the device kernel; the body executes 2*ceil(nrep/2) times."""
    s, W1a, W1b, W1c, b1p, madd, rowind, npos = _prep(**inputs)
    nc = _get_program(npos, with_loop=with_loop)
    maps = _in_maps(s, W1a, W1b, W1c, b1p, madd, rowind, nrep)
    res = run_bass_kernel_spmd(nc, maps, list(range(N_CORES)))
    return _gather(res.results)


def kernel(**inputs):
    return run(1, **inputs)


# revision 11
# speedup vs baseline: 1.0544x; 1.0544x over previous
"""Trainium2 Bass kernel for nn_DenseCoordination (gnn_message_passing).

Math (per batch b):
    hi = s @ W1a ; hj = s @ W1b                       [N, 2D]
    q[i,j,:] = (s_i * s_j) @ W1c + hi_i + hj_j + b1   [N, N, 2D]
    logits[i,j] = relu(q[i,j,:]) @ W2 + b2
    w = softmax(mask(logits), axis=-1) (nan_to_num)
    ctx = w @ s ; gate = ones

Sharding: 8 cores = 4 batches x 2 i-halves. Each core owns b = c//2 and
i in [128*(c%2), 128*(c%2)+128), computes its [128, N] logits / w / ctx.

v3 changes vs v2 baseline:
  - pt (si*sj) production moved from DVE to GPSIMD (Pool engine), freeing
    DVE for the consume.
  - Setup matmuls (hi/hj) in bf16 directly; no more f32r conversion copies.
  - Consume split: DVE does pos cols via max-trick stt (relu(H+hi) =
    max(H,-hi)+hi, the +hi row-constant cancels in softmax); ACT does neg
    cols via Relu+accum on H+hi (hi added by an identity matmul on PE).
  - Tail in bf16 (w transpose + ctx matmul with bf16 operands).
  - Body unrolled x2 inside For_i with bufs=2 pools so iteration t+1's
    input DMA + setup overlaps iteration t's j-loop / tail.
"""

import sys

sys.path.insert(0, "/opt/trn_rl_repo")

import numpy as np
import ml_dtypes

import concourse.bacc as bacc
import concourse.bass as bass
import concourse.tile as tile
from concourse import mybir
from concourse.bass_utils import run_bass_kernel_spmd

D = 256
N = 256
B = 4
H2 = 512  # 2*D
NI = 128  # i rows per core
N_CORES = 8
F32 = mybir.dt.float32
BF16 = mybir.dt.bfloat16
NEG_BIG = -1.0e30

_BUILD_CACHE: dict = {}


def _build(npos: int, with_loop: bool = True, variant: str = "full"):
    # variant: timing-attribution microbenchmarks (outputs wrong for != full)
    #   "full"     - the real kernel
    #   "nocons"   - j-loop without the DVE/ACT consume (accp/acca stale)
    #   "noonehot" - j-loop without the hj one-hot matmul
    #   "nopt"     - matmuls read a fixed pt tile (no per-j DVE production)
    AF = mybir.ActivationFunctionType
    ALU = mybir.AluOpType
    # Columns are permuted pos-first: [0:npos) pos, [npos:H2) neg.
    # ACT takes the SMALLER block (it needs hi pre-added by a PE identity
    # matmul, which costs PE cycles proportional to its width); DVE takes
    # the larger block via the max-trick stt (no hi add needed: the +hi
    # row-constant cancels in softmax).
    act_on_pos = npos <= H2 - npos
    if act_on_pos:
        aLo, aHi = 0, npos          # ACT: relu+accum over pos cols
        dLo, dHi = npos, H2         # DVE: max-trick over neg cols
    else:
        aLo, aHi = npos, H2         # ACT: relu+accum over neg cols
        dLo, dHi = 0, npos          # DVE: max-trick over pos cols
    cA = aHi - aLo
    cD = dHi - dLo

    nc = bacc.Bacc("TRN2", target_bir_lowering=False, debug=False,
                   num_devices=N_CORES)

    sT_in = nc.dram_tensor("sT", [D, N], F32, kind="ExternalInput").ap()
    sb_in = nc.dram_tensor("sb", [N, D], BF16, kind="ExternalInput").ap()
    sTb_in = nc.dram_tensor("sTb", [D, N], BF16, kind="ExternalInput").ap()
    sTib_in = nc.dram_tensor("sTib", [D, NI], BF16, kind="ExternalInput").ap()
    w1ab_in = nc.dram_tensor("W1ab", [D, H2], BF16, kind="ExternalInput").ap()
    w1bb_in = nc.dram_tensor("W1bb", [D, H2], BF16, kind="ExternalInput").ap()
    w1cb_in = nc.dram_tensor("W1cb", [D, H2], BF16, kind="ExternalInput").ap()
    b1b_in = nc.dram_tensor("b1b", [1, H2], BF16, kind="ExternalInput").ap()
    madd_in = nc.dram_tensor("madd", [NI, N], F32, kind="ExternalInput").ap()
    rowind_in = nc.dram_tensor("rowind", [NI, 1], F32, kind="ExternalInput").ap()
    identb_in = nc.dram_tensor("identb", [128, 128], BF16,
                               kind="ExternalInput").ap()
    nrep_in = nc.dram_tensor("nrep", [1, 1], mybir.dt.int32,
                             kind="ExternalInput").ap()
    w_out = nc.dram_tensor("w", [NI, N], F32, kind="ExternalOutput").ap()
    ctx_out = nc.dram_tensor("ctx", [NI, D], F32, kind="ExternalOutput").ap()

    with tile.TileContext(nc) as tc:
        with (
            tc.tile_pool(name="persist", bufs=2) as pp,
            tc.tile_pool(name="prod", bufs=6) as prodp,
            tc.tile_pool(name="trash", bufs=4) as trp,
            tc.tile_pool(name="psum", bufs=6, space="PSUM") as psp,
            tc.tile_pool(name="psum2", bufs=1, space="PSUM") as psp2,
            tc.tile_pool(name="psum3", bufs=1, space="PSUM") as psp3,
            tc.tile_pool(name="small", bufs=4) as smp,
        ):
            def body():
                # ---- load inputs into SBUF
                sT_sb = []
                sb_sb = []
                sTb_sb = []
                sTib_sb = []
                w1ab_sb = []
                w1bb_sb = []
                w1cb_sb = []
                for c in range(2):
                    t = pp.tile([128, N], F32, tag=f"sT{c}", name=f"sT{c}")
                    nc.sync.dma_start(t[:], sT_in[128 * c:128 * c + 128, :])
                    sT_sb.append(t)
                    t = pp.tile([128, D], BF16, tag=f"sb{c}", name=f"sb{c}")
                    nc.sync.dma_start(t[:], sb_in[128 * c:128 * c + 128, :])
                    sb_sb.append(t)
                    t = pp.tile([128, N], BF16, tag=f"sTb{c}", name=f"sTb{c}")
                    nc.sync.dma_start(t[:], sTb_in[128 * c:128 * c + 128, :])
                    sTb_sb.append(t)
                    t = pp.tile([128, NI], BF16, tag=f"sTib{c}", name=f"sTib{c}")
                    nc.sync.dma_start(t[:], sTib_in[128 * c:128 * c + 128, :])
                    sTib_sb.append(t)
                    for nm, src, lst in (("a", w1ab_in, w1ab_sb),
                                         ("b", w1bb_in, w1bb_sb),
                                         ("c", w1cb_in, w1cb_sb)):
                        t = pp.tile([128, H2], BF16, tag=f"W1{nm}{c}",
                                    name=f"W1{nm}{c}")
                        nc.sync.dma_start(t[:], src[128 * c:128 * c + 128, :])
                        lst.append(t)
                b1_sb = pp.tile([1, H2], BF16, tag="b1")
                nc.sync.dma_start(b1_sb[:], b1b_in[:])
                madd_sb = pp.tile([NI, N], F32, tag="madd")
                nc.sync.dma_start(madd_sb[:], madd_in[:])
                rowind_sb = pp.tile([NI, 1], F32, tag="rowind")
                nc.sync.dma_start(rowind_sb[:], rowind_in[:])
                identb_sb = pp.tile([128, 128], BF16, tag="identb")
                nc.sync.dma_start(identb_sb[:], identb_in[:])
                onesb_sb = pp.tile([1, 128], BF16, tag="onesb")
                nc.gpsimd.memset(onesb_sb[:], 1.0)

                # ---- setup: HJ' = s @ W1b' + b1'  (2 chunks of 128 j's)
                hj_bf = []
                for jc in range(2):
                    ps = psp.tile([128, H2], F32, tag="ps")
                    for kc in range(2):
                        nc.tensor.matmul(
                            ps[:], sTb_sb[kc][:, 128 * jc:128 * jc + 128],
                            w1bb_sb[kc][:], start=(kc == 0), stop=False)
                    nc.tensor.matmul(ps[:], onesb_sb[0:1, :], b1_sb[0:1, :],
                                     start=False, stop=True)
                    t = pp.tile([128, H2], BF16, tag=f"hjb{jc}", name=f"hjb{jc}")
                    nc.scalar.copy(t[:], ps[:])
                    hj_bf.append(t)

                # ---- setup: HI' = s[i-range] @ W1a' (no b1)
                ps = psp.tile([128, H2], F32, tag="ps")
                for kc in range(2):
                    nc.tensor.matmul(ps[:], sTib_sb[kc][:], w1ab_sb[kc][:],
                                     start=(kc == 0), stop=(kc == 1))
                hi_bf = pp.tile([128, H2], BF16, tag="hib")
                nc.scalar.copy(hi_bf[:], ps[:])
                neghi_sb = pp.tile([128, cD], F32, tag="neghi")
                nc.scalar.mul(neghi_sb[:], ps[:, dLo:dHi], -1.0)
                # (the +hi shift in the max-trick is constant per row i over
                #  the DVE block -> cancels in softmax)

                # ---- main fixed-j loop, software-pipelined.
                # pt production runs LA iterations ahead of the matmuls so
                # the (in-order) DVE never makes the PE wait for its
                # stationary operand: per-j DVE order is
                # [pt_{j+LA}, stt_j], not [pt_j, stt_j].
                accp = pp.tile([NI, N], F32, tag="accp")
                acca = pp.tile([NI, N], F32, tag="acca")
                LA = 3
                pt_ring = {}

                def emit_pt(j):
                    pts = []
                    for kc in range(2):
                        t = prodp.tile([128, NI], BF16, tag=f"pt{kc}",
                                       name=f"pt{kc}")
                        nc.vector.tensor_scalar_mul(
                            t[:], sTib_sb[kc][:],
                            sT_sb[kc][:, j:j + 1])
                        pts.append(t)
                    pt_ring[j] = pts

                if variant in ("nocons", "noonehot"):
                    nc.gpsimd.memset(accp[:], 0.0)
                    nc.gpsimd.memset(acca[:], 0.0)
                for j in range(LA):
                    if variant != "nopt":
                        emit_pt(j)
                if variant == "nopt":
                    emit_pt(0)
                    pt_fixed = pt_ring.pop(0)
                for j in range(N):
                    jc, jr = j // 128, j % 128
                    if variant != "nopt":
                        if j + LA < N:
                            emit_pt(j + LA)
                        pt = pt_ring.pop(j)
                    else:
                        pt = pt_fixed
                    ps = psp.tile([128, H2], F32, tag="ps")
                    for kc in range(2):
                        nc.tensor.matmul(
                            ps[:], pt[kc][:], w1cb_sb[kc][:],
                            start=(kc == 0), stop=False)
                    # += hj'_j + b1' on all cols (one-hot row jr broadcast)
                    if variant != "noonehot":
                        nc.tensor.matmul(
                            ps[:],
                            identb_sb[:, jr:jr + 1].to_broadcast((128, 128)),
                            hj_bf[jc][:],
                            start=False, stop=(cA == 0))
                    if cA > 0:
                        # += hi' on the ACT cols only
                        nc.tensor.matmul(
                            ps[:, aLo:aHi], identb_sb[:], hi_bf[:, aLo:aHi],
                            start=False, stop=True)
                        if variant != "nocons":
                            tr2 = trp.tile([128, cA], BF16, tag="tr2")
                            nc.scalar.activation(tr2[:], ps[:, aLo:aHi],
                                                 AF.Relu,
                                                 accum_out=acca[:, j:j + 1])
                    else:
                        nc.gpsimd.memset(acca[:, j:j + 1], 0.0)
                    if cD > 0 and variant != "nocons":
                        tr1 = trp.tile([128, cD], BF16, tag="tr1")
                        nc.vector.scalar_tensor_tensor(
                            out=tr1[:], in0=ps[:, dLo:dHi], scalar=0.0,
                            in1=neghi_sb[:], op0=ALU.add,
                            op1=ALU.max, accum_out=accp[:, j:j + 1])

                # logits: pos-block accumulator minus neg-block accumulator
                # (per-row-i constants cancel in softmax)
                logits = pp.tile([NI, N], F32, tag="logits")
                if act_on_pos:
                    nc.vector.tensor_sub(logits[:], acca[:], accp[:])
                else:
                    nc.vector.tensor_sub(logits[:], accp[:], acca[:])

                # ---- masked softmax over j
                l2 = pp.tile([NI, N], F32, tag="l2")
                nc.vector.tensor_add(l2[:], logits[:], madd_sb[:])
                negm = smp.tile([NI, 1], F32, tag="negm")
                nc.vector.tensor_reduce(negm[:], l2[:],
                                        axis=mybir.AxisListType.X, op=ALU.max,
                                        negate=True)
                ex = pp.tile([NI, N], F32, tag="ex")
                ssum = smp.tile([NI, 1], F32, tag="ssum")
                nc.scalar.activation(ex[:], l2[:], AF.Exp, bias=negm[:, 0:1],
                                     accum_out=ssum[:, 0:1])
                rec = smp.tile([NI, 1], F32, tag="rec")
                nc.vector.reciprocal(rec[:], ssum[:])
                rec2 = smp.tile([NI, 1], F32, tag="rec2")
                nc.vector.tensor_mul(rec2[:], rec[:], rowind_sb[:])
                w_sb = pp.tile([NI, N], F32, tag="wsb")
                nc.vector.tensor_scalar_mul(w_sb[:], ex[:], rec2[:, 0:1])
                wb_sb = pp.tile([NI, N], BF16, tag="wbsb")
                nc.vector.tensor_copy(wb_sb[:], w_sb[:])

                # ---- ctx = w @ s  (transpose w on the PE first, bf16)
                wt_sb = []
                for jc in range(2):
                    pst = psp3.tile([128, 128], BF16, tag="pst")
                    nc.tensor.transpose(pst[:],
                                        wb_sb[:, 128 * jc:128 * jc + 128],
                                        identb_sb[:])
                    t = smp.tile([128, 128], BF16, tag=f"wt{jc}", name=f"wt{jc}")
                    nc.vector.tensor_copy(t[:], pst[:])
                    wt_sb.append(t)
                psc = psp2.tile([128, D], F32, tag="tail")
                for jc in range(2):
                    nc.tensor.matmul(psc[:], wt_sb[jc][:], sb_sb[jc][:],
                                     start=(jc == 0), stop=(jc == 1))
                ctx_sb = pp.tile([NI, D], F32, tag="ctxsb")
                nc.scalar.copy(ctx_sb[:], psc[:])

                # ---- outputs
                nc.sync.dma_start(w_out[:], w_sb[:])
                nc.sync.dma_start(ctx_out[:], ctx_sb[:])

            if with_loop:
                nrep_sb = smp.tile([1, 1], mybir.dt.int32, tag="nrep")
                nc.sync.dma_start(nrep_sb[:], nrep_in[:])
                rv = nc.values_load(nrep_sb[0:1, 0:1], min_val=1,
                                    max_val=100000,
                                    skip_runtime_bounds_check=True)
                with tc.For_i(0, rv, 1):
                    body()
                    body()
            else:
                body()

    nc.compile()
    return nc


def _prep(s, W1, b1, W2, b2, adj_allowed, active_mask, act_mask):
    s = np.ascontiguousarray(np.asarray(s, dtype=np.float32))
    W1 = np.asarray(W1, dtype=np.float32)
    b1 = np.asarray(b1, dtype=np.float32).reshape(-1)
    W2 = np.asarray(W2, dtype=np.float32).reshape(-1)  # [2D]
    adj = np.asarray(adj_allowed)
    am = np.asarray(active_mask)
    km = np.asarray(act_mask)

    pos = W2 >= 0.0
    perm = np.concatenate([np.nonzero(pos)[0], np.nonzero(~pos)[0]])
    npos = int(pos.sum())
    w2p = np.abs(W2[perm])
    W1a = np.ascontiguousarray(W1[:D][:, perm] * w2p[None, :])
    W1b = np.ascontiguousarray(W1[D:2 * D][:, perm] * w2p[None, :])
    W1c = np.ascontiguousarray(W1[2 * D:][:, perm] * w2p[None, :])
    b1p = np.ascontiguousarray((b1[perm] * w2p)[None, :])

    valid = (adj > 0) & (am > 0)[:, None, :] & (km > 0)[:, :, None]
    madd = np.where(valid, np.float32(0.0), np.float32(NEG_BIG))
    rowind = valid.any(axis=-1).astype(np.float32)
    return s, W1a, W1b, W1c, b1p, madd, rowind, npos


def _in_maps(s, W1a, W1b, W1c, b1p, madd, rowind, nrep):
    # device trip count: body is unrolled x2 inside For_i
    trip = max(1, (int(nrep) + 1) // 2)
    nrep_arr = np.full((1, 1), trip, dtype=np.int32)
    bf = ml_dtypes.bfloat16
    w1ab = W1a.astype(bf)
    w1bb = W1b.astype(bf)
    w1cb = W1c.astype(bf)
    b1b = b1p.astype(bf)
    identb = np.eye(128, dtype=np.float32).astype(bf)
    maps = []
    for c in range(N_CORES):
        b, i0 = c // 2, NI * (c % 2)
        sb = s[b]
        sTb = np.ascontiguousarray(sb.T)
        sTb_bf = sTb.astype(bf)
        maps.append({
            "sT": sTb,
            "sb": sb.astype(bf),
            "sTb": sTb_bf,
            "sTib": np.ascontiguousarray(sTb_bf[:, i0:i0 + NI]),
            "W1ab": w1ab, "W1bb": w1bb, "W1cb": w1cb, "b1b": b1b,
            "madd": np.ascontiguousarray(madd[b, i0:i0 + NI]),
            "rowind": np.ascontiguousarray(rowind[b, i0:i0 + NI, None]),
            "identb": identb,
            "nrep": nrep_arr,
        })
    return maps


def _gather(results):
    w = np.empty((B, N, N), dtype=np.float32)
    ctx = np.empty((B, N, D), dtype=np.float32)
    for c in range(N_CORES):
        b, i0 = c // 2, NI * (c % 2)
        w[b, i0:i0 + NI] = results[c]["w"]
        ctx[b, i0:i0 + NI] = results[c]["ctx"]
    gate = np.ones((B, N, N), dtype=np.float32)
    return ctx, gate, w


def _get_program(npos, with_loop=True, variant="full"):
    key = (npos, with_loop, variant)
    if key not in _BUILD_CACHE:
        _BUILD_CACHE[key] = _build(npos, with_loop=with_loop,
                                   variant=variant)
    return _BUILD_CACHE[key]


def run(nrep, *, with_loop=True, **inputs):
    """Run the device kernel; the body executes 2*ceil(nrep/2) times."""
    s, W1a, W1b, W1c, b1p, madd, rowind, npos = _prep(**inputs)
    nc = _get_program(npos, with_loop=with_loop)
    maps = _in_maps(s, W1a, W1b, W1c, b1p, madd, rowind, nrep)
    res = run_bass_kernel_spmd(nc, maps, list(range(N_CORES)))
    return _gather(res.results)


def kernel(**inputs):
    return run(1, **inputs)
